# revision 28
# baseline (speedup 1.0000x reference)
"""Trainium2 Bass kernel for nn_Circuit: batched 3-qubit circuit.

Circuit per state (8-dim complex, B=2^21 independent states):
  H on qubits 0,1 -> RX(theta0) on q0, RX(theta1) on q1 -> CNOT(q0 -> q2).
The whole circuit is one 8x8 complex unitary U(theta); the kernel applies
y = U @ x per state and emits (B, 8, 2) with real/imag stacked last.

The problem is HBM-bandwidth-bound (cost model: all DMA traffic shares one
360 B/ns bus), so I/O is fp16 — the 2e-2 error gate leaves orders of
magnitude of headroom (fp16 I/O lands ~7e-4):

  - host packs x_real/x_imag into one interleaved fp16 tensor
    xq[s, 0:8] = Re x[s], xq[s, 8:16] = Im x[s] (32 B per state), so a
    [128 x 128] fp16 block of a natural-layout tile holds 8 whole states
  - per iteration (32768 states, 8 groups of 4 blocks): 7 groups arrive
    via a plain DMA and are PE-transposed (128x128 fp16, cheapest per
    byte); 1 group arrives via the XBAR transposing DMA directly into the
    matmul operand buffer.  The XBAR group leads each iteration's matmul
    phase, giving the PSUM->SBUF copies of the PE-transposed groups a
    head start (7/1 is the measured optimum: plain DMA is 364 ns/group vs
    XBAR 448, but all-plain overloads the copy engines)
  - one 128x128 fp16 matmul per block against a host-built block-diagonal
    gate matrix BD (8 states x 16 components per block; real/imag of U
    interleaved so a single matmul does the whole complex matvec per
    block); fp32 PSUM accumulation
  - PSUM->SBUF out-copies are split DVE/ACT (Pool cannot access PSUM);
    contiguous fp16 DMA out; host unpacks fp16 (S,16) -> fp32 (B,8,2)

fp16 PSUM note: PSUM is cell-addressed (one 32-bit cell per fp16), so a
[128, 512]-fp16 transpose group occupies a FULL 2KB bank; tp tensors are
declared [128, 1024] fp16 (byte-sized to one bank) and only cols 0:512
are used, in a 4-bank ring. 4 tp banks + 4 f32 matmul banks = all 8.

Latency trims (each validated against the cost model):
  - in/out DMAs split in halves (split_in / split_out=2): deeper DMA
    queue occupancy hides per-DMA DGE latencies; the first out-half
    launches as soon as the copies covering groups h0..3 land
  - consts off the SP ring (consts="split"): the identity is generated
    on the otherwise-idle Pool engine (memset + affine_select) and BD
    arrives via the ACT ring, so the first input DMA issues immediately
    after the program preamble

Pipeline (per core, 8 iterations, 50678 ns on the TimelineSim cost model
vs a ~49.7 us structural bound = 47.5 us bus content + preamble/final
latencies; baseline fp32 kernel was 102777 ns):
  SP :  2 plain in-DMA halves + XBAR in-DMA per iter (3-deep rings)
  PE :  [matmuls of iter i-1][transposes of iter i] per slot
  DVE:  7 tq copies (fp16 2x mode) + 3 out-copies per iter
  ACT:  5 out-copies + 2 out-DMA halves per iter; BD const DMA
  Pool: identity generation at startup
"""

import contextlib

import numpy as np

import concourse.bass as bass
import concourse.mybir as mybir
from concourse.bass_utils import run_bass_kernel_spmd

F16 = mybir.dt.float16
F32 = mybir.dt.float32

B = 2097152            # total batch
N_CORES = 8
S_CORE = B // N_CORES  # states per core = 262144
W = 256                # states per partition-row per iteration
N_IT = S_CORE // (128 * W)  # 8 iterations
NPL = 7                # groups per iter via plain DMA + PE transpose (of 8)
RING = 3               # input ring depth
DVE_OUT = (2, 4, 6)    # out-copy slots (M mod 8) on DVE
POOL_OUT = ()          # Pool cannot access PSUM (BIR verifier); ACT takes the rest


def circuit_unitary(theta):
    """8x8 complex unitary of the whole circuit, component index 4a+2b+c
    for qubits (a, b, c) = (q0, q1, q2)."""
    theta = np.asarray(theta, np.float64)
    inv_sqrt2 = 1.0 / np.sqrt(2.0)
    H = np.array([[1.0, 1.0], [1.0, -1.0]], np.complex128) * inv_sqrt2
    I2 = np.eye(2, dtype=np.complex128)

    def rx(t):
        c, s = np.cos(t / 2.0), np.sin(t / 2.0)
        return np.array([[c, -1j * s], [-1j * s, c]], np.complex128)

    A0 = rx(theta[0]) @ H
    A1 = rx(theta[1]) @ H
    G = np.kron(A0, np.kron(A1, I2))
    # CNOT control q0, target q2: out[a,b,c] = in[a,b,c^a]
    U = np.empty_like(G)
    for a in range(2):
        for b_ in range(2):
            for c in range(2):
                U[4 * a + 2 * b_ + c, :] = G[4 * a + 2 * b_ + (c ^ a), :]
    return U


def build_bd(theta):
    """Block-diagonal gate operand BD [128, 128] fp16.

    Contraction row r = 16*w + j (w in 0..7 = state within block,
    j in 0..7 -> Re x_j, j in 8..15 -> Im x_{j-8}); output column
    n = 16*w + 2*k2 + ri (component k2, ri = 0 real / 1 imag).
    A transposed input block T (T[r, c] = state c's component j) gives
    out[c, n] = sum_r T[r, c] * BD[r, n] = the full complex matvec."""
    U = circuit_unitary(theta)
    Ur = U.real.astype(np.float32)
    Ui = U.imag.astype(np.float32)
    BD = np.zeros((128, 128), np.float32)
    for w in range(8):
        r0 = 16 * w
        for k in range(8):
            for k2 in range(8):
                col = r0 + 2 * k2
                BD[r0 + k, col + 0] = Ur[k2, k]
                BD[r0 + k, col + 1] = Ui[k2, k]
                BD[r0 + 8 + k, col + 0] = -Ui[k2, k]
                BD[r0 + 8 + k, col + 1] = Ur[k2, k]
    return BD.astype(np.float16)


def build_nc(n_it=N_IT, w=W, npl=NPL, r=RING, dve_out=DVE_OUT, pool_out=POOL_OUT,
             out_eng="act", interleave=False, split_out=False, split_in=False,
             consts="sp", in_eng="sp", mm_first=False, xbar_eng="sp",
             sin_g=None, sout_g=None):
    nc = bass.Bass("TRN2", target_bir_lowering=False, debug=False)
    s = n_it * 128 * w
    fw = w * 16                 # fp16 per partition-row per iter (4096)
    pw = npl * 512              # fp16 cols of the plain region
    # matmul emission order per iter: the XBAR group(s) first, or (mm_first)
    # the first-out-piece groups h0..3 first so piece 1 can launch early
    if npl == 8:
        mm_order = list(range(8))
    elif mm_first:
        mm_order = [0, 1, 2, 3] + list(range(npl, 8)) + list(range(4, npl))
    else:
        mm_order = list(range(npl, 8)) + list(range(npl))

    # out-copy engine assignment (by M mod 8) + per-engine ordinals
    eng_of, ord_of = {}, {}
    cv = ca = cp_ = 0
    for M in range(8 * n_it):
        k = M % 8
        if k in dve_out:
            cv += 1
            eng_of[M], ord_of[M] = "v", cv
        elif k in pool_out:
            cp_ += 1
            eng_of[M], ord_of[M] = "p", cp_
        else:
            ca += 1
            eng_of[M], ord_of[M] = "a", ca
    nv_per, np_per = len(dve_out), len(pool_out)
    na_per = 8 - nv_per - np_per

    xq = nc.dram_tensor("xq", [s, 16], F16, kind="ExternalInput").ap()
    bd = nc.dram_tensor("bd", [128, 128], F16, kind="ExternalInput").ap()
    idn = nc.dram_tensor("idn", [128, 128], F16, kind="ExternalInput").ap()
    out = nc.dram_tensor("out", [s, 16], F16, kind="ExternalOutput").ap()

    xq_v = xq.rearrange("(n p v) j -> n p (v j)", n=n_it, p=128, v=w)
    out_v = out.rearrange("(n p v) j -> n p (v j)", n=n_it, p=128, v=w)

    with contextlib.ExitStack() as ctx:
        ent = ctx.enter_context
        block = ent(nc.Block())
        s_const = ent(nc.semaphore("s_const"))
        s_inp = [ent(nc.semaphore(f"s_inp{j}")) for j in range(r)]
        s_inx = [ent(nc.semaphore(f"s_inx{j}")) for j in range(r)]
        s_pt = ent(nc.semaphore("s_pt"))   # +1 per transpose group
        s_pm = ent(nc.semaphore("s_pm"))   # +1 per matmul group
        s_tq = ent(nc.semaphore("s_tq"))   # +1 per tq copy (DVE)
        s_ov = ent(nc.semaphore("s_ov"))   # out copies on DVE
        s_oa = ent(nc.semaphore("s_oa"))   # out copies on ACT
        s_op = ent(nc.semaphore("s_op"))   # out copies on Pool
        s_out = [ent(nc.semaphore(f"s_out{j}")) for j in range(3)]
        ident = ent(nc.sbuf_tensor("ident", [128, 128], F16))
        bd_sb = ent(nc.sbuf_tensor("bd_sb", [128, 128], F16))
        xq_sb = [ent(nc.sbuf_tensor(f"xq{j}", [128, pw], F16)) for j in range(r)]
        tq_sb = [ent(nc.sbuf_tensor(f"tq{j}", [128, fw], F16)) for j in range(r)]
        ot_sb = [ent(nc.sbuf_tensor(f"ot{j}", [128, fw], F16)) for j in range(3)]
        # fp16 PSUM is cell-addressed (one 32-bit cell per element), so a
        # transpose group [128, 512] fp16 occupies a FULL bank; declare
        # [128, 1024] (byte-sized to one bank) and use only cols 0:512.
        # 4-bank ring over the global transpose-group counter.
        tp_ps = [ent(nc.psum_tensor(f"tp{j}", [128, 1024], F16)) for j in range(4)]
        po_ps = [ent(nc.psum_tensor(f"po{j}", [128, 512], F32)) for j in range(4)]

        def tp_slot(t):
            return tp_ps[t % 4].ap()[:, 0:512]

        def wait_po(ring, M):
            # po[M%4] free once the out-copy of M-4 (same engine) completed
            if M >= 4:
                Mp = M - 4
                sem = {"v": s_ov, "a": s_oa, "p": s_op}[eng_of[Mp]]
                ring.wait_ge(sem, ord_of[Mp])

        NS = int(split_out) if split_out else 1

        def out_dma(ring, j):
            gh = 8 // NS       # h-groups per piece
            bounds = [0, sout_g, 8] if (NS == 2 and sout_g) else [q * gh for q in range(NS)] + [8]
            for q in range(NS):
                h_lo, h_hi = bounds[q], bounds[q + 1]
                # all copies covering positions up to the furthest needed
                need = [p for p in range(8) if mm_order[p] < h_hi]
                nv_h = max(
                    (ord_of[8 * j + p] for p in need if eng_of[8 * j + p] == "v"),
                    default=0,
                )
                na_h = max(
                    (ord_of[8 * j + p] for p in need if eng_of[8 * j + p] == "a"),
                    default=0,
                )
                if nv_h:
                    ring.wait_ge(s_ov, nv_h)
                if na_h:
                    ring.wait_ge(s_oa, na_h)
                c0, c1 = 512 * h_lo, 512 * h_hi
                ring.dma_start(
                    out_v[j][:, c0:c1], ot_sb[j % 3].ap()[:, c0:c1]
                ).then_inc(s_out[j % 3], 16)

        def emit_xbar(ring, i):
            if i >= r:
                # tq xbar region free: first mm group of iter i-r done
                ring.wait_ge(s_pm, 8 * (i - r) + (8 - npl))
            tq3 = (
                tq_sb[i % r]
                .ap()[:, pw:fw]
                .rearrange("p (b c) -> p b c", b=(8 - npl) * 4)
            )
            ring.dma_start_transpose(tq3, xq_v[i][:, pw:fw]).then_inc(
                s_inx[i % r], 16
            )

        sg = sin_g if sin_g is not None else npl // 2

        def emit_plain_in(ring, i):
            if split_in:
                hp = sg * 512
                ring.dma_start(
                    xq_sb[i % r].ap()[:, 0:hp], xq_v[i][:, 0:hp]
                ).then_inc(s_inp[i % r], 16)
                ring.dma_start(
                    xq_sb[i % r].ap()[:, hp:pw], xq_v[i][:, hp:pw]
                ).then_inc(s_inp[i % r], 16)
            else:
                ring.dma_start(
                    xq_sb[i % r].ap(), xq_v[i][:, 0:pw]
                ).then_inc(s_inp[i % r], 16)

        @block.sync
        def _(sync):
            if consts == "sp":
                sync.dma_start(bd_sb.ap(), bd).then_inc(s_const, 16)
                sync.dma_start(ident.ap(), idn).then_inc(s_const, 16)
            for i in range(n_it):
                if out_eng == "sp" and i >= 1:
                    out_dma(sync, i - 1)
                if in_eng == "sp":
                    if i >= r:
                        # xq slot free: transposes of iter i-r done
                        sync.wait_ge(s_pt, npl * (i - r + 1))
                    emit_plain_in(sync, i)
                if npl < 8 and xbar_eng == "sp":
                    emit_xbar(sync, i)
            if out_eng == "sp":
                out_dma(sync, n_it - 1)

        @block.tensor
        def _(tensor):
            iap = ident.ap()

            def tr_one(i, h):
                xs = xq_sb[i % r].ap()
                T = npl * i + h
                if split_in:
                    if h == 0:
                        tensor.wait_ge(s_inp[i % r], 32 * (i // r) + 16)
                    elif h == sg:
                        tensor.wait_ge(s_inp[i % r], 32 * (i // r) + 32)
                elif h == 0:
                    tensor.wait_ge(s_inp[i % r], 16 * (i // r + 1))
                if T >= 4:
                    # tp bank T%4 freed by the tq-copy of group T-4
                    ip, hp_ = divmod(T - 4, npl)
                    if npl == 8 and hp_ == 7:
                        tensor.wait_ge(s_op, ip + 1)
                    elif npl == 8:
                        tensor.wait_ge(s_tq, 7 * ip + hp_ + 1)
                    else:
                        tensor.wait_ge(s_tq, T - 3)
                tp = tp_slot(T)
                for b4 in range(4):
                    c0 = 128 * (4 * h + b4)
                    tr = nc.tensor.transpose(
                        tp[:, 128 * b4 : 128 * b4 + 128], xs[:, c0 : c0 + 128], iap
                    )
                tr.then_inc(s_pt, 1)

            def trs(i):
                for h in range(npl):
                    tr_one(i, h)

            def mm_one(i, pos):
                tq = tq_sb[i % r].ap()
                h = mm_order[pos]
                M = 8 * i + pos
                if h == npl:
                    tensor.wait_ge(s_inx[i % r], 16 * (i // r + 1))
                if npl == 8:
                    if h == 7:
                        tensor.wait_ge(s_op, i + 1)
                    else:
                        tensor.wait_ge(s_tq, 7 * i + h + 1)
                elif h < npl:
                    tensor.wait_ge(s_tq, npl * i + h + 1)  # its copy done
                wait_po(tensor, M)
                pp = po_ps[M % 4].ap()
                for b4 in range(4):
                    mm = nc.tensor.matmul(
                        pp[:, 128 * b4 : 128 * b4 + 128],
                        tq[:, 512 * h + 128 * b4 : 512 * h + 128 * b4 + 128],
                        bd_sb.ap(),
                        start=True,
                        stop=True,
                    )
                mm.then_inc(s_pm, 1)

            def mms(i):
                for pos in range(8):
                    mm_one(i, pos)

            def slot(i):
                if interleave == "edge":
                    # trs 0..3 of iter i, all mms of i-1, trs 4.. of iter i:
                    # matmuls start after only 4 transposes; the trailing
                    # transposes feed copies consumed a slot later.
                    for h in range(min(4, npl)):
                        tr_one(i, h)
                    if i >= 1:
                        mms(i - 1)
                    for h in range(4, npl):
                        tr_one(i, h)
                    return
                if not interleave or i == 0:
                    if i >= 1:
                        mms(i - 1)
                    trs(i)
                    return
                # fine interleave: tr(i,h) / mm(i-1,pos) alternating
                tr_it = iter(range(npl))
                mm_it = iter(range(8))
                for k in range(npl + 8):
                    if k % 2 == 0:
                        h = next(tr_it, None)
                        if h is not None:
                            tr_one(i, h)
                        else:
                            mm_one(i - 1, next(mm_it))
                    else:
                        p = next(mm_it, None)
                        if p is not None:
                            mm_one(i - 1, p)
                        else:
                            tr_one(i, next(tr_it))

            tensor.wait_ge(s_const, 32)
            for i in range(n_it):
                slot(i)
            mms(n_it - 1)

        first_pos = {
            eng: min(p for p in range(8) if eng_of[p] == eng)
            for eng in set(eng_of[p] for p in range(8))
        }

        def out_copies(ring, cp, i, eng, s_o):
            for pos in range(8):
                M = 8 * i + pos
                if eng_of[M] != eng:
                    continue
                h = mm_order[pos]
                if pos == first_pos[eng] and i >= 3:
                    ring.wait_ge(s_out[i % 3], 16 * NS * (i // 3))
                ring.wait_ge(s_pm, M + 1)
                cp(
                    ot_sb[i % 3].ap()[:, 512 * h : 512 * h + 512],
                    po_ps[M % 4].ap(),
                ).then_inc(s_o, 1)

        @block.vector
        def _(vector):
            for i in range(n_it):
                if i >= 1:
                    out_copies(vector, nc.vector.tensor_copy, i - 1, "v", s_ov)
                for h in range(min(npl, 7)):
                    T = npl * i + h
                    vector.wait_ge(s_pt, T + 1)
                    if i >= r:
                        vector.wait_ge(s_pm, 8 * (i - r + 1))  # tq region free
                    nc.vector.tensor_copy(
                        tq_sb[i % r].ap()[:, 512 * h : 512 * h + 512], tp_slot(T)
                    ).then_inc(s_tq, 1)
            out_copies(vector, nc.vector.tensor_copy, n_it - 1, "v", s_ov)

        if consts == "split" or out_eng == "pool" or in_eng == "pool":
            @block.gpsimd
            def _(gpsimd):
                if consts == "split":
                    from concourse import masks
                    masks.make_identity(nc, ident.ap())
                    nc.gpsimd.memset(ident.ap()[0:1, 0:1], 1.0).then_inc(s_const, 16)
                for i in range(n_it):
                    if in_eng == "pool":
                        if i >= r:
                            gpsimd.wait_ge(s_pt, npl * (i - r + 1))
                        emit_plain_in(gpsimd, i)
                    if out_eng == "pool":
                        if i >= 1:
                            out_dma(gpsimd, i - 1)
                if out_eng == "pool":
                    out_dma(gpsimd, n_it - 1)

        @block.scalar
        def _(scalar):
            if consts == "split":
                scalar.dma_start(bd_sb.ap(), bd).then_inc(s_const, 16)

            def tq7(i):
                T = 8 * i + 7
                scalar.wait_ge(s_pt, T + 1)
                if i >= r:
                    scalar.wait_ge(s_pm, 8 * (i - r + 1))
                scalar.copy(
                    tq_sb[i % r].ap()[:, 512 * 7 : 512 * 8], tp_slot(T)
                ).then_inc(s_op, 1)

            if npl == 8:
                tq7(0)
            for i in range(n_it):
                if i >= 1:
                    j = i - 1
                    if npl == 8 and i >= 2:
                        tq7(j)
                    out_copies(scalar, nc.scalar.copy, j, "a", s_oa)
                    if out_eng == "act":
                        out_dma(scalar, j)
            j = n_it - 1
            if npl == 8:
                tq7(j)
            out_copies(scalar, nc.scalar.copy, j, "a", s_oa)
            if out_eng == "act":
                out_dma(scalar, j)

    return nc


_NC_CACHE = {}


def get_nc():
    if "nc" not in _NC_CACHE:
        _NC_CACHE["nc"] = build_nc(split_out=2, split_in=True, consts="split")
    return _NC_CACHE["nc"]


def kernel(x_real, x_imag, theta, angle=None, **_unused):
    x_real = np.asarray(x_real)
    x_imag = np.asarray(x_imag)
    theta = np.asarray(theta, np.float32)
    assert x_real.shape == (B, 8), x_real.shape

    xq = np.empty((B, 16), np.float16)
    xq[:, 0:8] = x_real
    xq[:, 8:16] = x_imag
    BD = build_bd(theta)
    eye = np.eye(128, dtype=np.float16)
    nc = get_nc()

    in_maps = []
    for c in range(N_CORES):
        sl = slice(c * S_CORE, (c + 1) * S_CORE)
        in_maps.append({"xq": xq[sl], "bd": BD, "idn": eye})

    res = run_bass_kernel_spmd(nc, in_maps, core_ids=list(range(N_CORES)))
    out16 = np.concatenate([r["out"] for r in res.results], axis=0)
    return out16.astype(np.float32).reshape(B, 8, 2)
